# revision 17
# baseline (speedup 1.0000x reference)
"""Trainium2 Bass kernel for nn_DotMatrix.

Math: for each (b, ell, t) the reference computes a complex pairwise dot
matrix O[i,j] = sum_m z[i,m] * w[j,m] where z = rep[b,:,t,:,:] as complex
and w the sign-flipped conjugation partner.  As a real matmul:

  lhsT[k, i]   k = (c,m) stacked: [Zr.T; Zi.T]                 [2m, 256]
  rhs[k, 2j+c'] c'=0: [FZr; -FZi], c'=1: [FZi; FZr]            [2m, 512]
  out = lhsT.T @ rhs  -> [256 i, 512 (j,c)]

with FZr[m',j] = s[m'] * Zr[j, M-1-m'], s[m'] = (-1)^(ell+m').

Precision: fp16 operands with fp32 PSUM accumulation and fp16 stores
give ~4e-4 relative error on the final output (gate is 2e-2).

Symmetry trick: the pairwise matrix is symmetric in (i,j) for both
components, so each channel computes only 32-row i-blocks against
j >= 32*bi (56.25% of the matrix); the host mirrors the rest.

Sharding: 8 cores = 2 batches x 4 tau-quarters; each core owns 32
channels ch = ell*8 + s.  Four channels (a quad) share each matmul's
PSUM partitions via column tiling.

Input layout (the part that sets the critical path): each ell lives in
its own 32-aligned partition group (ell0@0, ell1@32, ell2@64, ell3@96),
so the four input loads land on disjoint SBUF AXI ports and stream
CONCURRENTLY — packed low-partition layouts funnel every load through
the same 1-2 ports and serialize (~6us).  Each ell is further split
into two per-quad tiles so quad (0,0) is gated only by its own 24KB
slice, not the whole ell tensor.  Eight dummy matmuls bridge the input
wait so the PE's HAM clock gate is warm when real work starts.

PSUM pack per quad: i-blocks (0,1,7) in a 2-bank tile, (2,6), (3,5),
(4) in one bank each — every i-block inside a single 2KB bank — so
evacuation is one wide copy per tile (f32 -> fp16), split ScalarE
{1024+256 cols} / VectorE {512+512}.  Each quad's output leaves as two
stores (1024 cols as soon as the A-tile copy lands, remainder after)
on alternating HWDGE rings.  Host reassembles [2,256,256,128,2].
"""

import numpy as np

import concourse.bacc as bacc
import concourse.mybir as mybir
from concourse.bass_utils import run_bass_kernel_spmd
from concourse.tile import TileContext

B, N, TAU, NELL = 2, 256, 32, 4
NCORES = 8
F32 = mybir.dt.float32
F16 = mybir.dt.float16
KS = [2 * (2 * ell + 1) for ell in range(NELL)]       # 2, 6, 10, 14
# contraction dims padded with zero rows to a multiple of 4 so moving
# data stays 8B-aligned per column
KP = [4, 8, 12, 16]
RG = {0: 0, 1: 32, 2: 64, 3: 96}                      # partition group per ell
BIW = [512 - 64 * bi for bi in range(8)]              # cols per 32-row i-block
# PSUM pack: per quad, two psB tiles (1 bank each): (3,5) and (2,6),
# plus one 3-bank psA tile holding (0) | (1,7) | (4) — every i-block
# inside a single 2KB bank.  Evacuation is exactly two copies per quad:
# vector (25ns fixed + ~1.05ns/col floor) takes the two 512-col psB
# CASTs (~1.4us), scalar (260ns fixed + 0.83ns/col) the single
# 1280-col psA copy (~1.35us) — both under the 1.72us/quad store pace.
BIO2 = {3: 0, 5: 320, 2: 512, 6: 896, 0: 1024, 1: 1536, 7: 1984, 4: 2048}
OTW = 2304                                            # sum of all widths
# quads ordered by when their input slice lands (small/early ells first)
QUAD_ORDER = [(0, 0), (0, 1), (2, 0), (1, 0), (2, 1), (1, 1), (3, 0), (3, 1)]
IN_COLS = 4 * 256 + 4 * 512                           # 3072: 4 lhs slots then 4 rhs slots

_NC_CACHE = {}


def _build_bass():
    nc = bacc.Bacc()
    # One input tensor per (ell, quad-half): [KP, 3072] fp16; cols
    # [0:1024) hold four 256-wide lhsT slots, [1024:3072) four 512-wide
    # rhs slots.
    inps = {}
    for e in range(NELL):
        for v in range(2):
            inps[(e, v)] = nc.declare_dram_parameter(
                f"inp{e}_{v}", [KP[e], IN_COLS], F16, isOutput=False
            )
    out = nc.declare_dram_parameter("out", [128, 8 * OTW], F16, isOutput=True)

    with TileContext(nc) as tc:
        with (
            tc.tile_pool(name="lin", bufs=1) as lin_pool,
            tc.tile_pool(name="i00", bufs=1) as p00,
            tc.tile_pool(name="i01", bufs=1) as p01,
            tc.tile_pool(name="i10", bufs=1) as p10,
            tc.tile_pool(name="i11", bufs=1) as p11,
            tc.tile_pool(name="i20", bufs=1) as p20,
            tc.tile_pool(name="i21", bufs=1) as p21,
            tc.tile_pool(name="i30", bufs=1) as p30,
            tc.tile_pool(name="i31", bufs=1) as p31,
            tc.tile_pool(name="psA", bufs=2, space="PSUM") as psA_pool,
            tc.tile_pool(name="psB1", bufs=1, space="PSUM") as psB1_pool,
            tc.tile_pool(name="psB2", bufs=1, space="PSUM") as psB2_pool,
            tc.tile_pool(name="ot1", bufs=4) as ot1_pool,
        ):
            in_pools = {(0, 0): p00, (0, 1): p01, (1, 0): p10, (1, 1): p11,
                        (2, 0): p20, (2, 1): p21, (3, 0): p30, (3, 1): p31}
            in_sbs = {
                k: in_pools[k].tile([128, IN_COLS], F16, name=f"in_sb{k[0]}_{k[1]}")
                for k in in_pools
            }
            # Input loads first so descriptor generation starts
            # immediately; rings alternate, issue order matches quad
            # consumption order.  Port-disjoint partition groups let the
            # transfers run concurrently.
            # transfers on one ring run in FIFO order, and an ell's two
            # halves share SBUF ports anyway — so group each ell's halves
            # back-to-back and put port-heavy ell3 behind tiny ell0 on
            # sync, ell1/ell2 on scalar.  Every quad's slice lands >1.5us
            # before the store stream needs it.
            for (e, v), eng in (
                ((0, 0), nc.sync), ((2, 0), nc.scalar),
                ((0, 1), nc.sync), ((2, 1), nc.scalar),
                ((3, 0), nc.sync), ((1, 0), nc.scalar),
                ((3, 1), nc.sync), ((1, 1), nc.scalar),
            ):
                eng.dma_start(
                    out=in_sbs[(e, v)][RG[e] : RG[e] + KP[e], :],
                    in_=inps[(e, v)][:],
                )
            # PE pre-warm: dependency-free dummy matmuls keep the PE busy
            # from kernel start until the first input lands, so the HAM
            # clock gate is released before real matmuls stream.  The
            # memset goes on the otherwise-idle GpSimd so the warmups
            # start immediately.  warm_ps shares psC's single bank; the
            # last warmup retires right around when the first input's
            # completion semaphore fires, so sharing costs nothing.
            warm_in = lin_pool.tile([128, 512], F16, name="warm_in")
            warm_ps = psB1_pool.tile([128, 512], F32, tag="psB1", name="warm_ps")
            nc.gpsimd.memset(warm_in[:], 0.0)
            for _ in range(8):
                nc.tensor.matmul(
                    warm_ps[:, 0:512], warm_in[:, 0:128], warm_in[:, 0:512],
                    start=True, stop=True,
                )
            def mm_block(in_sb, bp, K, ps, bi, poff):
                W = BIW[bi]
                for c4 in range(4):
                    lo = c4 * 256
                    ro = 1024 + c4 * 512
                    nc.tensor.matmul(
                        ps[c4 * 32 : (c4 + 1) * 32, poff : poff + W],
                        in_sb[bp : bp + K, lo + bi * 32 : lo + bi * 32 + 32],
                        in_sb[bp : bp + K, ro + 64 * bi : ro + 512],
                        start=True,
                        stop=True,
                        tile_position=(bp, c4 * 32),
                    )

            for qidx, (e, v) in enumerate(QUAD_ORDER):
                K = KP[e]
                bp = RG[e]
                in_sb = in_sbs[(e, v)]
                ot = ot1_pool.tile([128, OTW], F16)
                ob = qidx * OTW
                pb1 = psB1_pool.tile([128, 512], F32, tag="psB1", name="psb1")
                mm_block(in_sb, bp, K, pb1, 3, 0)
                mm_block(in_sb, bp, K, pb1, 5, 320)
                pb2 = psB2_pool.tile([128, 512], F32, tag="psB2", name="psb2")
                mm_block(in_sb, bp, K, pb2, 2, 0)
                mm_block(in_sb, bp, K, pb2, 6, 384)
                pa = psA_pool.tile([128, 1536], F32, name="psa")
                mm_block(in_sb, bp, K, pa, 0, 0)
                mm_block(in_sb, bp, K, pa, 1, 512)
                mm_block(in_sb, bp, K, pa, 7, 960)
                mm_block(in_sb, bp, K, pa, 4, 1024)
                nc.vector.tensor_copy(out=ot[:, 0:512], in_=pb1[:, 0:512])
                # the first two quads prime the store stream in 512-col
                # pieces as soon as each CAST lands; afterwards piece-1 is
                # one 262KB store per quad
                if qidx < 2:
                    nc.sync.dma_start(out=out[:, ob : ob + 512], in_=ot[:, 0:512])
                nc.vector.tensor_copy(out=ot[:, 512:1024], in_=pb2[:, 0:512])
                if qidx < 2:
                    nc.sync.dma_start(
                        out=out[:, ob + 512 : ob + 1024], in_=ot[:, 512:1024]
                    )
                else:
                    nc.sync.dma_start(out=out[:, ob : ob + 1024], in_=ot[:, 0:1024])
                nc.scalar.copy(ot[:, 1024:OTW], pa[:, 0:1280])
                # piece-2 alternates between the scalar HWDGE ring (its
                # descgen chains right behind scalar's own copy) and the
                # gpsimd SWDGE ring — three rings together hold the
                # ~344GB/s HBM write rate with each far under its own cap
                seng = nc.scalar if qidx % 2 == 0 else nc.gpsimd
                seng.dma_start(
                    out=out[:, ob + 1024 : ob + OTW], in_=ot[:, 1024:OTW]
                )
    nc.compile()
    return nc


def _host_prep(reps, cid):
    """Build per-core fp16 lhsT/rhs input tensors (one per ell, quad)."""
    b, tq = cid // 4, cid % 4
    im = {}
    for ell in range(NELL):
        rep = reps[ell]
        m = 2 * ell + 1
        s_vec = ((-1.0) ** (ell + np.arange(m))).astype(np.float32)
        for v in range(2):
            arr = np.zeros((KP[ell], IN_COLS), np.float32)
            for c4 in range(4):
                t = tq * 8 + v * 4 + c4
                Z = rep[b, :, t]                      # [256, m, 2]
                Zr, Zi = Z[..., 0], Z[..., 1]         # [256, m]
                arr[0:m, c4 * 256 : c4 * 256 + 256] = Zr.T
                arr[m : 2 * m, c4 * 256 : c4 * 256 + 256] = Zi.T
                FZr = s_vec[:, None] * Zr[:, ::-1].T             # [m, 256]
                FZi = s_vec[:, None] * Zi[:, ::-1].T
                R = np.empty((2 * m, 256, 2), np.float32)
                R[0:m, :, 0] = FZr
                R[m:, :, 0] = -FZi
                R[0:m, :, 1] = FZi
                R[m:, :, 1] = FZr
                ro = 1024 + c4 * 512
                arr[0 : 2 * m, ro : ro + 512] = R.reshape(2 * m, 512)
            im[f"inp{ell}_{v}"] = arr.astype(np.float16)
    return im


def _run(in_maps, **kw):
    if "nc" not in _NC_CACHE:
        _NC_CACHE["nc"] = _build_bass()
    return run_bass_kernel_spmd(_NC_CACHE["nc"], in_maps, list(range(NCORES)), **kw)


def kernel(rep0, rep1, rep2, rep3, _bass_kw=None):
    reps = [np.ascontiguousarray(np.asarray(r, dtype=np.float32)) for r in (rep0, rep1, rep2, rep3)]
    in_maps = [_host_prep(reps, cid) for cid in range(NCORES)]
    res = _run(in_maps, **(_bass_kw or {}))
    out = np.empty((B, N, N, NELL * TAU, 2), np.float32)
    for cid in range(NCORES):
        b, tq = cid // 4, cid % 4
        arr = res.results[cid]["out"]          # [128, 8*2304] fp16
        o = np.empty((NELL, 8, 256, 256, 2), np.float32)   # [ell, slot, i, j, c]
        for qidx, (e, v) in enumerate(QUAD_ORDER):
            a = arr[:, qidx * OTW : (qidx + 1) * OTW].astype(np.float32)
            for bi in range(8):
                nj = 256 - 32 * bi
                blk = a[:, BIO2[bi] : BIO2[bi] + BIW[bi]].reshape(4, 32, nj, 2)
                for c4 in range(4):
                    o[e, 4 * v + c4, 32 * bi : 32 * bi + 32, 32 * bi :, :] = blk[c4]
        for bi in range(1, 8):                  # mirror lower block triangle
            r = slice(32 * bi, 32 * bi + 32)
            o[:, :, r, : 32 * bi, :] = o[:, :, : 32 * bi, r, :].transpose(0, 1, 3, 2, 4)
        for e in range(NELL):
            lo = e * TAU + tq * 8
            out[b, :, :, lo : lo + 8, :] = o[e].transpose(1, 2, 0, 3)
    kernel.last_result = res
    return out


# revision 18
# speedup vs baseline: 1.0399x; 1.0399x over previous
"""Trainium2 Bass kernel for nn_DotMatrix.

Math: for each (b, ell, t) the reference computes a complex pairwise dot
matrix O[i,j] = sum_m z[i,m] * w[j,m] where z = rep[b,:,t,:,:] as complex
and w the sign-flipped conjugation partner.  As a real matmul:

  lhsT[k, i]   k = (c,m) stacked: [Zr.T; Zi.T]                 [2m, 256]
  rhs[k, 2j+c'] c'=0: [FZr; -FZi], c'=1: [FZi; FZr]            [2m, 512]
  out = lhsT.T @ rhs  -> [256 i, 512 (j,c)]

with FZr[m',j] = s[m'] * Zr[j, M-1-m'], s[m'] = (-1)^(ell+m').

Precision: fp16 operands with fp32 PSUM accumulation and fp16 stores
give ~4e-4 relative error on the final output (gate is 2e-2).

Symmetry trick: the pairwise matrix is symmetric in (i,j) for both
components, so each channel computes only 32-row i-blocks against
j >= 32*bi (56.25% of the matrix); the host mirrors the rest.

Sharding: 8 cores = 2 batches x 4 tau-quarters; each core owns 32
channels ch = ell*8 + s.  Four channels (a quad) share each matmul's
PSUM partitions via column tiling.

Input layout (the part that sets the critical path): each ell lives in
its own 32-aligned partition group (ell0@0, ell1@32, ell2@64, ell3@96),
so the four input loads land on disjoint SBUF AXI ports and stream
CONCURRENTLY — packed low-partition layouts funnel every load through
the same 1-2 ports and serialize (~6us).  Each ell is further split
into two per-quad tiles so quad (0,0) is gated only by its own 24KB
slice, not the whole ell tensor.  Eight dummy matmuls bridge the input
wait so the PE's HAM clock gate is warm when real work starts.

PSUM pack per quad: i-blocks (0,1,7) in a 2-bank tile, (2,6), (3,5),
(4) in one bank each — every i-block inside a single 2KB bank — so
evacuation is one wide copy per tile (f32 -> fp16), split ScalarE
{1024+256 cols} / VectorE {512+512}.  Each quad's output leaves as two
stores (1024 cols as soon as the A-tile copy lands, remainder after)
on alternating HWDGE rings.  Host reassembles [2,256,256,128,2].
"""

import numpy as np

import concourse.bacc as bacc
import concourse.mybir as mybir
from concourse.bass_utils import run_bass_kernel_spmd
from concourse.tile import TileContext

B, N, TAU, NELL = 2, 256, 32, 4
NCORES = 8
F32 = mybir.dt.float32
F16 = mybir.dt.float16
KS = [2 * (2 * ell + 1) for ell in range(NELL)]       # 2, 6, 10, 14
# contraction dims padded with zero rows to a multiple of 4 so moving
# data stays 8B-aligned per column
KP = [4, 8, 12, 16]
RG = {0: 0, 1: 32, 2: 64, 3: 96}                      # partition group per ell
BIW = [512 - 64 * bi for bi in range(8)]              # cols per 32-row i-block
# PSUM pack: per quad, two psB tiles (1 bank each): (3,5) and (2,6),
# plus one 3-bank psA tile holding (0) | (1,7) | (4) — every i-block
# inside a single 2KB bank.  Evacuation is exactly two copies per quad:
# vector (25ns fixed + ~1.05ns/col floor) takes the two 512-col psB
# CASTs (~1.4us), scalar (260ns fixed + 0.83ns/col) the single
# 1280-col psA copy (~1.35us) — both under the 1.72us/quad store pace.
BIO2 = {3: 0, 5: 320, 2: 512, 6: 896, 0: 1024, 1: 1536, 7: 1984, 4: 2048}
OTW = 2304                                            # sum of all widths
# quads ordered by when their input slice lands (small/early ells first)
QUAD_ORDER = [(0, 0), (0, 1), (2, 0), (1, 0), (2, 1), (1, 1), (3, 0), (3, 1)]
IN_COLS = 4 * 256 + 4 * 512                           # 3072: 4 lhs slots then 4 rhs slots

_NC_CACHE = {}


def _build_bass():
    nc = bacc.Bacc()
    # One input tensor per (ell, quad-half): [KP, 3072] fp16; cols
    # [0:1024) hold four 256-wide lhsT slots, [1024:3072) four 512-wide
    # rhs slots.
    inps = {}
    for e in range(NELL):
        for v in range(2):
            inps[(e, v)] = nc.declare_dram_parameter(
                f"inp{e}_{v}", [KP[e], IN_COLS], F16, isOutput=False
            )
    out = nc.declare_dram_parameter("out", [128, 8 * OTW], F16, isOutput=True)

    with TileContext(nc) as tc:
        with (
            tc.tile_pool(name="lin", bufs=1) as lin_pool,
            tc.tile_pool(name="i00", bufs=1) as p00,
            tc.tile_pool(name="i01", bufs=1) as p01,
            tc.tile_pool(name="i10", bufs=1) as p10,
            tc.tile_pool(name="i11", bufs=1) as p11,
            tc.tile_pool(name="i20", bufs=1) as p20,
            tc.tile_pool(name="i21", bufs=1) as p21,
            tc.tile_pool(name="i30", bufs=1) as p30,
            tc.tile_pool(name="i31", bufs=1) as p31,
            tc.tile_pool(name="psA", bufs=2, space="PSUM") as psA_pool,
            tc.tile_pool(name="psB1", bufs=1, space="PSUM") as psB1_pool,
            tc.tile_pool(name="psB2", bufs=1, space="PSUM") as psB2_pool,
            tc.tile_pool(name="ot1", bufs=4) as ot1_pool,
        ):
            in_pools = {(0, 0): p00, (0, 1): p01, (1, 0): p10, (1, 1): p11,
                        (2, 0): p20, (2, 1): p21, (3, 0): p30, (3, 1): p31}
            in_sbs = {
                k: in_pools[k].tile([128, IN_COLS], F16, name=f"in_sb{k[0]}_{k[1]}")
                for k in in_pools
            }
            # Input loads first so descriptor generation starts
            # immediately; rings alternate, issue order matches quad
            # consumption order.  Port-disjoint partition groups let the
            # transfers run concurrently.
            # transfers on one ring run in FIFO order, and an ell's two
            # halves share SBUF ports anyway — so group each ell's halves
            # back-to-back and put port-heavy ell3 behind tiny ell0 on
            # sync, ell1/ell2 on scalar.  Every quad's slice lands >1.5us
            # before the store stream needs it.
            for (e, v), eng in (
                ((0, 0), nc.sync), ((2, 0), nc.scalar),
                ((0, 1), nc.sync), ((2, 1), nc.scalar),
                ((3, 0), nc.sync), ((1, 0), nc.scalar),
                ((3, 1), nc.sync), ((1, 1), nc.scalar),
            ):
                eng.dma_start(
                    out=in_sbs[(e, v)][RG[e] : RG[e] + KP[e], :],
                    in_=inps[(e, v)][:],
                )
            # PE pre-warm: dependency-free dummy matmuls keep the PE busy
            # from kernel start until the first input lands, so the HAM
            # clock gate is released before real matmuls stream.  The
            # memset goes on the otherwise-idle GpSimd so the warmups
            # start immediately.  warm_ps shares psC's single bank; the
            # last warmup retires right around when the first input's
            # completion semaphore fires, so sharing costs nothing.
            warm_in = lin_pool.tile([128, 512], F16, name="warm_in")
            warm_ps = psB1_pool.tile([128, 512], F32, tag="psB1", name="warm_ps")
            nc.gpsimd.memset(warm_in[:], 0.0)
            for _ in range(8):
                nc.tensor.matmul(
                    warm_ps[:, 0:512], warm_in[:, 0:128], warm_in[:, 0:512],
                    start=True, stop=True,
                )
            def mm_block(in_sb, bp, K, ps, bi, poff):
                W = BIW[bi]
                for c4 in range(4):
                    lo = c4 * 256
                    ro = 1024 + c4 * 512
                    nc.tensor.matmul(
                        ps[c4 * 32 : (c4 + 1) * 32, poff : poff + W],
                        in_sb[bp : bp + K, lo + bi * 32 : lo + bi * 32 + 32],
                        in_sb[bp : bp + K, ro + 64 * bi : ro + 512],
                        start=True,
                        stop=True,
                        tile_position=(bp, c4 * 32),
                    )

            for qidx, (e, v) in enumerate(QUAD_ORDER):
                K = KP[e]
                bp = RG[e]
                in_sb = in_sbs[(e, v)]
                ot = ot1_pool.tile([128, OTW], F16)
                ob = qidx * OTW
                # psA blocks first: psA is the only double-buffered pool,
                # so the PE enters each new quad with zero cross-engine
                # waits — by the time it reaches the single-buffered psB
                # tiles, the previous quad's CASTs are a full quad old.
                # The serial chain (PE -> copy -> PE) that paced earlier
                # variants at ~1.9us/quad is gone; stores pace instead.
                pa = psA_pool.tile([128, 1536], F32, name="psa")
                mm_block(in_sb, bp, K, pa, 0, 0)
                mm_block(in_sb, bp, K, pa, 1, 512)
                mm_block(in_sb, bp, K, pa, 7, 960)
                mm_block(in_sb, bp, K, pa, 4, 1024)
                pb1 = psB1_pool.tile([128, 512], F32, tag="psB1", name="psb1")
                mm_block(in_sb, bp, K, pb1, 3, 0)
                mm_block(in_sb, bp, K, pb1, 5, 320)
                pb2 = psB2_pool.tile([128, 512], F32, tag="psB2", name="psb2")
                mm_block(in_sb, bp, K, pb2, 2, 0)
                mm_block(in_sb, bp, K, pb2, 6, 384)
                # piece-2 (psA cols) leaves first: the first two quads
                # split scalar's copy so a small piece primes the stream
                if qidx < 2:
                    nc.scalar.copy(ot[:, 1024:1536], pa[:, 0:512])
                    nc.scalar.dma_start(
                        out=out[:, ob + 1024 : ob + 1536], in_=ot[:, 1024:1536]
                    )
                    nc.scalar.copy(ot[:, 1536:OTW], pa[:, 512:1280])
                    nc.scalar.dma_start(
                        out=out[:, ob + 1536 : ob + OTW], in_=ot[:, 1536:OTW]
                    )
                else:
                    nc.scalar.copy(ot[:, 1024:OTW], pa[:, 0:1280])
                    # alternate the scalar HWDGE ring (descgen chains right
                    # behind scalar's own copy) and the gpsimd SWDGE ring
                    seng = nc.scalar if qidx % 2 == 0 else nc.gpsimd
                    seng.dma_start(
                        out=out[:, ob + 1024 : ob + OTW], in_=ot[:, 1024:OTW]
                    )
                nc.vector.tensor_copy(out=ot[:, 0:512], in_=pb1[:, 0:512])
                nc.vector.tensor_copy(out=ot[:, 512:1024], in_=pb2[:, 0:512])
                nc.sync.dma_start(out=out[:, ob : ob + 1024], in_=ot[:, 0:1024])
    nc.compile()
    return nc


def _host_prep(reps, cid):
    """Build per-core fp16 lhsT/rhs input tensors (one per ell, quad)."""
    b, tq = cid // 4, cid % 4
    im = {}
    for ell in range(NELL):
        rep = reps[ell]
        m = 2 * ell + 1
        s_vec = ((-1.0) ** (ell + np.arange(m))).astype(np.float32)
        for v in range(2):
            arr = np.zeros((KP[ell], IN_COLS), np.float32)
            for c4 in range(4):
                t = tq * 8 + v * 4 + c4
                Z = rep[b, :, t]                      # [256, m, 2]
                Zr, Zi = Z[..., 0], Z[..., 1]         # [256, m]
                arr[0:m, c4 * 256 : c4 * 256 + 256] = Zr.T
                arr[m : 2 * m, c4 * 256 : c4 * 256 + 256] = Zi.T
                FZr = s_vec[:, None] * Zr[:, ::-1].T             # [m, 256]
                FZi = s_vec[:, None] * Zi[:, ::-1].T
                R = np.empty((2 * m, 256, 2), np.float32)
                R[0:m, :, 0] = FZr
                R[m:, :, 0] = -FZi
                R[0:m, :, 1] = FZi
                R[m:, :, 1] = FZr
                ro = 1024 + c4 * 512
                arr[0 : 2 * m, ro : ro + 512] = R.reshape(2 * m, 512)
            im[f"inp{ell}_{v}"] = arr.astype(np.float16)
    return im


def _run(in_maps, **kw):
    if "nc" not in _NC_CACHE:
        _NC_CACHE["nc"] = _build_bass()
    return run_bass_kernel_spmd(_NC_CACHE["nc"], in_maps, list(range(NCORES)), **kw)


def kernel(rep0, rep1, rep2, rep3, _bass_kw=None):
    reps = [np.ascontiguousarray(np.asarray(r, dtype=np.float32)) for r in (rep0, rep1, rep2, rep3)]
    in_maps = [_host_prep(reps, cid) for cid in range(NCORES)]
    res = _run(in_maps, **(_bass_kw or {}))
    out = np.empty((B, N, N, NELL * TAU, 2), np.float32)
    for cid in range(NCORES):
        b, tq = cid // 4, cid % 4
        arr = res.results[cid]["out"]          # [128, 8*2304] fp16
        o = np.empty((NELL, 8, 256, 256, 2), np.float32)   # [ell, slot, i, j, c]
        for qidx, (e, v) in enumerate(QUAD_ORDER):
            a = arr[:, qidx * OTW : (qidx + 1) * OTW].astype(np.float32)
            for bi in range(8):
                nj = 256 - 32 * bi
                blk = a[:, BIO2[bi] : BIO2[bi] + BIW[bi]].reshape(4, 32, nj, 2)
                for c4 in range(4):
                    o[e, 4 * v + c4, 32 * bi : 32 * bi + 32, 32 * bi :, :] = blk[c4]
        for bi in range(1, 8):                  # mirror lower block triangle
            r = slice(32 * bi, 32 * bi + 32)
            o[:, :, r, : 32 * bi, :] = o[:, :, : 32 * bi, r, :].transpose(0, 1, 3, 2, 4)
        for e in range(NELL):
            lo = e * TAU + tq * 8
            out[b, :, :, lo : lo + 8, :] = o[e].transpose(1, 2, 0, 3)
    kernel.last_result = res
    return out
